# revision 9
# baseline (speedup 1.0000x reference)
"""Trainium2 Bass kernel for the AAGC layer (gnn_message_passing).

Math: M = sigmoid-chain(tiny weights) @ A_cur is a 15x15 mixing matrix;
out = sigmoid(einsum("ij,bjf->bif", M, x)) over B=524288 samples of
15 joints x 9 features. Memory-bound: 283MB in + 283MB out.

Strategy (pure data parallel over 8 cores):
- Host re-lays each core's shard so that every SBUF tile is
  [120 partitions, 9216 f32] with partition p = s*15 + j holding 8
  interleaved samples' joint-rows; each partition's bytes are contiguous
  in DRAM, so DMAs run at full HBM rate.
- Device computes M from the tiny replicated weights (a chain of small
  matmuls/sigmoids on TensorE/ScalarE), builds W = blockdiag_8(M^T)
  [120x120] once, then streams: DMA-in -> matmul(W as stationary) ->
  ScalarE sigmoid -> DMA-out. One matmul mixes the 15 joint-rows of 8
  samples at a time across partitions; free dim is chunked at 512
  (fp32 moving-operand limit), grouped x3 per PSUM tile so each
  activation instruction covers 1536 columns.
"""

import numpy as np

import concourse.bass as bass
import concourse.bacc as bacc
import concourse.mybir as mybir
import concourse.tile as tile
from concourse.bass_utils import run_bass_kernel_spmd

N_CORES = 8
B = 524288
J = 15          # joints
F = 9           # features per joint
FEAT = J * F    # 135
S = 8           # samples interleaved per partition block
P = S * J       # 120 partitions used
SPC = B // N_CORES   # 65536 samples per core
G = 8                # DRAM tiles per core
T = SPC // (G * S)   # 1024 free-chunks per tile
COLS = T * F         # 9216 f32 per partition per tile
CHUNK = 512          # fp32 matmul moving free-dim limit
GROUP = 3            # matmul chunks per PSUM tile / activation
NGROUP = COLS // (CHUNK * GROUP)  # 6
H = 50          # hidden width of the tiny weight chain

FP32 = mybir.dt.float32
AF = mybir.ActivationFunctionType

# Set by test.py to profile; harness default is a plain fast run.
TRACE = False
TRACE_KWARGS = {}

_CACHE = {}


def build_nc(debug=False, n_tiles=G, repeats=1):
    nc = bacc.Bacc("TRN2", target_bir_lowering=False, debug=debug)

    x = nc.dram_tensor("x", [n_tiles * P, COLS], FP32, kind="ExternalInput").ap()
    y = nc.dram_tensor("y", [n_tiles * P, COLS], FP32, kind="ExternalOutput").ap()
    a_init = nc.dram_tensor("a_init", [J, J], FP32, kind="ExternalInput").ap()
    a_change = nc.dram_tensor("a_change", [J, J], FP32, kind="ExternalInput").ap()
    hidden = nc.dram_tensor("hidden", [J, H], FP32, kind="ExternalInput").ap()
    sigma = nc.dram_tensor("sigma", [H, H], FP32, kind="ExternalInput").ap()
    kern = nc.dram_tensor("kern", [H, J], FP32, kind="ExternalInput").ap()
    bias_w = nc.dram_tensor("bias_w", [J, H], FP32, kind="ExternalInput").ap()

    with tile.TileContext(nc) as tc:
        with tc.tile_pool(name="const", bufs=1) as cp:
            # --- tiny replicated weights ---
            a_init_t = cp.tile([J, J], FP32)
            nc.sync.dma_start(a_init_t[:], a_init[:])
            a_change_t = cp.tile([J, J], FP32)
            nc.sync.dma_start(a_change_t[:], a_change[:])
            hidden_t = cp.tile([J, H], FP32)
            nc.sync.dma_start(hidden_t[:], hidden[:])
            sigma_t = cp.tile([H, H], FP32)
            nc.sync.dma_start(sigma_t[:], sigma[:])
            kern_t = cp.tile([H, J], FP32)
            nc.sync.dma_start(kern_t[:], kern[:])
            bias_t = cp.tile([J, H], FP32)
            nc.sync.dma_start(bias_t[:], bias_w[:])

            # identity_15 for TensorE transposes of [15, *] tiles
            ones_t = cp.tile([J, J], FP32)
            nc.gpsimd.memset(ones_t[:], 1.0)
            id15 = cp.tile([J, J], FP32)
            nc.gpsimd.affine_select(
                id15[:], ones_t[:], pattern=[[1, J]], base=0,
                channel_multiplier=-1,
                compare_op=mybir.AluOpType.is_equal, fill=0.0,
            )

            with tc.tile_pool(name="pre_psum", bufs=2,
                              space=bass.MemorySpace.PSUM) as pp:

                def transpose15(src, p_out, tag):
                    # src is [15, p_out]; returns SBUF [p_out, 15] = src.T
                    ps = pp.tile([p_out, J], FP32, tag="pre_t")
                    nc.tensor.transpose(ps[:], src[:], id15[:])
                    dst = cp.tile([p_out, J], FP32, tag=tag)
                    nc.vector.tensor_copy(dst[:], ps[:])
                    return dst

                # A_cur = A_init + A_change
                acur = cp.tile([J, J], FP32)
                nc.vector.tensor_add(acur[:], a_init_t[:], a_change_t[:])
                acur_T = transpose15(acur, J, "acur_T")

                # support = sigmoid(A_cur @ Hidden)       [15, 50]
                sup_ps = pp.tile([J, H], FP32, tag="pre_mm")
                nc.tensor.matmul(sup_ps[:], acur_T[:], hidden_t[:])
                support = cp.tile([J, H], FP32)
                nc.scalar.activation(support[:], sup_ps[:], AF.Sigmoid)
                support_T = transpose15(support, H, "support_T")

                # Hidden_new = sigmoid(support @ sigma + bias)   [15, 50]
                hn_ps = pp.tile([J, H], FP32, tag="pre_mm")
                nc.tensor.matmul(hn_ps[:], support_T[:], sigma_t[:])
                hn_pre = cp.tile([J, H], FP32)
                nc.vector.tensor_add(hn_pre[:], hn_ps[:], bias_t[:])
                hn = cp.tile([J, H], FP32)
                nc.scalar.activation(hn[:], hn_pre[:], AF.Sigmoid)
                hn_T = transpose15(hn, H, "hn_T")

                # mapfuc = sigmoid(Hidden_new @ kernel)   [15, 15]
                mf_ps = pp.tile([J, J], FP32, tag="pre_mm")
                nc.tensor.matmul(mf_ps[:], hn_T[:], kern_t[:])
                mapfuc = cp.tile([J, J], FP32)
                nc.scalar.activation(mapfuc[:], mf_ps[:], AF.Sigmoid)
                mapfuc_T = transpose15(mapfuc, J, "mapfuc_T")

                # M = mapfuc @ A_cur                      [15, 15]
                m_ps = pp.tile([J, J], FP32, tag="pre_mm")
                nc.tensor.matmul(m_ps[:], mapfuc_T[:], acur[:])
                m_sb = cp.tile([J, J], FP32)
                nc.vector.tensor_copy(m_sb[:], m_ps[:])
                m_T = transpose15(m_sb, J, "m_T")

            # W = blockdiag_8(M^T)  [120, 120]; stationary operand so that
            # matmul out = W.T @ rhs applies M to each sample's 15 rows.
            w_sb = cp.tile([P, P], FP32)
            nc.gpsimd.memset(w_sb[:], 0.0)
            for s in range(S):
                nc.sync.dma_start(
                    w_sb[s * J:(s + 1) * J, s * J:(s + 1) * J], m_T[:]
                )

            # --- main streaming loop ---
            import os
            xin_bufs = int(os.environ.get("XIN_BUFS", "2"))
            yout_bufs = int(os.environ.get("YOUT_BUFS", "2"))
            dma_split = int(os.environ.get("DMA_SPLIT", "1"))
            alt_rings = int(os.environ.get("ALT_RINGS", "0"))
            with (
                tc.tile_pool(name="xin", bufs=xin_bufs) as xin_p,
                tc.tile_pool(name="yout", bufs=yout_bufs) as yout_p,
                tc.tile_pool(name="mm_psum", bufs=2,
                             space=bass.MemorySpace.PSUM) as mm_pp,
            ):
                def eng(i):
                    if not alt_rings:
                        return nc.sync, nc.scalar
                    return ((nc.sync, nc.scalar) if i % 2 == 0
                            else (nc.scalar, nc.sync))

                for i, g in enumerate(
                        [g for _ in range(repeats) for g in range(n_tiles)]):
                    if alt_rings == 2:
                        rings = (nc.sync, nc.scalar)
                        in_eng = out_eng = None
                    else:
                        in_eng, out_eng = eng(i)
                    xt = xin_p.tile([P, COLS], FP32)
                    step = COLS // dma_split
                    for d in range(dma_split):
                        e = rings[d % 2] if alt_rings == 2 else in_eng
                        e.dma_start(
                            xt[:, d * step:(d + 1) * step],
                            x[g * P:(g + 1) * P, d * step:(d + 1) * step])
                    if int(os.environ.get("COPY_ONLY", "0")):
                        for d in range(dma_split):
                            e = rings[(d + 1) % 2] if alt_rings == 2 else out_eng
                            e.dma_start(
                                y[g * P:(g + 1) * P, d * step:(d + 1) * step],
                                xt[:, d * step:(d + 1) * step])
                        continue
                    yt = yout_p.tile([P, COLS], FP32)
                    for h in range(NGROUP):
                        ps = mm_pp.tile([P, GROUP * CHUNK], FP32)
                        for c in range(GROUP):
                            lo = (h * GROUP + c) * CHUNK
                            nc.tensor.matmul(
                                ps[:, c * CHUNK:(c + 1) * CHUNK],
                                w_sb[:],
                                xt[:, lo:lo + CHUNK],
                            )
                        nc.scalar.activation(
                            yt[:, h * GROUP * CHUNK:(h + 1) * GROUP * CHUNK],
                            ps[:], AF.Sigmoid,
                        )
                    for d in range(dma_split):
                        e = rings[(d + 1) % 2] if alt_rings == 2 else out_eng
                        e.dma_start(
                            y[g * P:(g + 1) * P, d * step:(d + 1) * step],
                            yt[:, d * step:(d + 1) * step])

    nc.compile()
    return nc


def shard_inputs(inputs):
    """Host-side prep: per-core x re-layout + replicated tiny weights."""
    nf = np.ascontiguousarray(np.asarray(inputs["new_features"], dtype=np.float32))
    small = {
        "a_init": np.ascontiguousarray(np.asarray(inputs["A_init"], np.float32)),
        "a_change": np.ascontiguousarray(np.asarray(inputs["A_change"], np.float32)),
        "hidden": np.ascontiguousarray(np.asarray(inputs["Hidden"], np.float32)),
        "sigma": np.ascontiguousarray(np.asarray(inputs["sigma"], np.float32)),
        "kern": np.ascontiguousarray(np.asarray(inputs["kernel"], np.float32)),
        "bias_w": np.ascontiguousarray(np.asarray(inputs["bias"], np.float32)),
    }
    in_maps = []
    for c in range(N_CORES):
        shard = nf[c * SPC:(c + 1) * SPC]
        xc = np.ascontiguousarray(
            shard.reshape(G, T, S, J, F).transpose(0, 2, 3, 1, 4)
        ).reshape(G * P, COLS)
        in_maps.append({"x": xc, **small})
    return in_maps


def unshard_output(results):
    outs = []
    for c in range(N_CORES):
        yc = results[c]["y"]
        out = np.ascontiguousarray(
            yc.reshape(G, S, J, T, F).transpose(0, 3, 1, 2, 4)
        ).reshape(SPC, FEAT)
        outs.append(out)
    return np.concatenate(outs, axis=0)


def kernel(**inputs):
    if "nc" not in _CACHE:
        _CACHE["nc"] = build_nc()
    nc = _CACHE["nc"]
    in_maps = shard_inputs(inputs)
    res = run_bass_kernel_spmd(
        nc, in_maps, core_ids=list(range(N_CORES)),
        trace=TRACE, **TRACE_KWARGS,
    )
    _CACHE["last_result"] = res
    return unshard_output(res.results)


# revision 10
# speedup vs baseline: 1.4931x; 1.4931x over previous
"""Trainium2 Bass kernel for the AAGC layer (gnn_message_passing).

Math: M = sigmoid-chain(tiny weights) @ A_cur is a 15x15 mixing matrix;
out = sigmoid(einsum("ij,bjf->bif", M, x)) over B=524288 samples of
15 joints x 9 features. Memory-bound: 283MB in + 283MB out.

Strategy (pure data parallel over 8 cores):
- Host re-lays each core's shard so that every SBUF tile is
  [120 partitions, 9216 f32] with partition p = s*15 + j holding 8
  interleaved samples' joint-rows; each partition's bytes are contiguous
  in DRAM, so DMAs run at full HBM rate.
- Device computes M from the tiny replicated weights (a chain of small
  matmuls/sigmoids on TensorE/ScalarE), builds W = blockdiag_8(M^T)
  [120x120] once, then streams: DMA-in -> matmul(W as stationary) ->
  ScalarE sigmoid -> DMA-out. One matmul mixes the 15 joint-rows of 8
  samples at a time across partitions; free dim is chunked at 512
  (fp32 moving-operand limit), grouped x3 per PSUM tile so each
  activation instruction covers 1536 columns.
"""

import numpy as np

import concourse.bass as bass
import concourse.bacc as bacc
import concourse.mybir as mybir
import concourse.tile as tile
from concourse.bass_utils import run_bass_kernel_spmd

N_CORES = 8
B = 524288
J = 15          # joints
F = 9           # features per joint
FEAT = J * F    # 135
S = 8           # samples interleaved per partition block
P = S * J       # 120 partitions used
SPC = B // N_CORES   # 65536 samples per core
G = 8                # DRAM tiles per core
T = SPC // (G * S)   # 1024 free-chunks per tile
COLS = T * F         # 9216 f32 per partition per tile
CHUNK = 512          # fp32 matmul moving free-dim limit
GROUP = 3            # matmul chunks per PSUM tile / activation
NGROUP = COLS // (CHUNK * GROUP)  # 6
H = 50          # hidden width of the tiny weight chain

FP32 = mybir.dt.float32
AF = mybir.ActivationFunctionType

# Set by test.py to profile; harness default is a plain fast run.
TRACE = False
TRACE_KWARGS = {}

_CACHE = {}


def build_nc(debug=False, n_tiles=G, repeats=1):
    nc = bacc.Bacc("TRN2", target_bir_lowering=False, debug=debug)

    x = nc.dram_tensor("x", [n_tiles * P, COLS], FP32, kind="ExternalInput").ap()
    y = nc.dram_tensor("y", [n_tiles * P, COLS], FP32, kind="ExternalOutput").ap()
    a_init = nc.dram_tensor("a_init", [J, J], FP32, kind="ExternalInput").ap()
    a_change = nc.dram_tensor("a_change", [J, J], FP32, kind="ExternalInput").ap()
    hidden = nc.dram_tensor("hidden", [J, H], FP32, kind="ExternalInput").ap()
    sigma = nc.dram_tensor("sigma", [H, H], FP32, kind="ExternalInput").ap()
    kern = nc.dram_tensor("kern", [H, J], FP32, kind="ExternalInput").ap()
    bias_w = nc.dram_tensor("bias_w", [J, H], FP32, kind="ExternalInput").ap()

    with tile.TileContext(nc) as tc:
        with tc.tile_pool(name="const", bufs=1) as cp:
            # --- tiny replicated weights ---
            a_init_t = cp.tile([J, J], FP32)
            nc.sync.dma_start(a_init_t[:], a_init[:])
            a_change_t = cp.tile([J, J], FP32)
            nc.sync.dma_start(a_change_t[:], a_change[:])
            hidden_t = cp.tile([J, H], FP32)
            nc.sync.dma_start(hidden_t[:], hidden[:])
            sigma_t = cp.tile([H, H], FP32)
            nc.sync.dma_start(sigma_t[:], sigma[:])
            kern_t = cp.tile([H, J], FP32)
            nc.sync.dma_start(kern_t[:], kern[:])
            bias_t = cp.tile([J, H], FP32)
            nc.sync.dma_start(bias_t[:], bias_w[:])

            # identity_15 for TensorE transposes of [15, *] tiles
            ones_t = cp.tile([J, J], FP32)
            nc.gpsimd.memset(ones_t[:], 1.0)
            id15 = cp.tile([J, J], FP32)
            nc.gpsimd.affine_select(
                id15[:], ones_t[:], pattern=[[1, J]], base=0,
                channel_multiplier=-1,
                compare_op=mybir.AluOpType.is_equal, fill=0.0,
            )

            with tc.tile_pool(name="pre_psum", bufs=2,
                              space=bass.MemorySpace.PSUM) as pp:

                def transpose15(src, p_out, tag):
                    # src is [15, p_out]; returns SBUF [p_out, 15] = src.T
                    ps = pp.tile([p_out, J], FP32, tag="pre_t")
                    nc.tensor.transpose(ps[:], src[:], id15[:])
                    dst = cp.tile([p_out, J], FP32, tag=tag)
                    nc.vector.tensor_copy(dst[:], ps[:])
                    return dst

                # A_cur = A_init + A_change
                acur = cp.tile([J, J], FP32)
                nc.vector.tensor_add(acur[:], a_init_t[:], a_change_t[:])
                acur_T = transpose15(acur, J, "acur_T")

                # support = sigmoid(A_cur @ Hidden)       [15, 50]
                sup_ps = pp.tile([J, H], FP32, tag="pre_mm")
                nc.tensor.matmul(sup_ps[:], acur_T[:], hidden_t[:])
                support = cp.tile([J, H], FP32)
                nc.scalar.activation(support[:], sup_ps[:], AF.Sigmoid)
                support_T = transpose15(support, H, "support_T")

                # Hidden_new = sigmoid(support @ sigma + bias)   [15, 50]
                hn_ps = pp.tile([J, H], FP32, tag="pre_mm")
                nc.tensor.matmul(hn_ps[:], support_T[:], sigma_t[:])
                hn_pre = cp.tile([J, H], FP32)
                nc.vector.tensor_add(hn_pre[:], hn_ps[:], bias_t[:])
                hn = cp.tile([J, H], FP32)
                nc.scalar.activation(hn[:], hn_pre[:], AF.Sigmoid)
                hn_T = transpose15(hn, H, "hn_T")

                # mapfuc = sigmoid(Hidden_new @ kernel)   [15, 15]
                mf_ps = pp.tile([J, J], FP32, tag="pre_mm")
                nc.tensor.matmul(mf_ps[:], hn_T[:], kern_t[:])
                mapfuc = cp.tile([J, J], FP32)
                nc.scalar.activation(mapfuc[:], mf_ps[:], AF.Sigmoid)
                mapfuc_T = transpose15(mapfuc, J, "mapfuc_T")

                # M = mapfuc @ A_cur                      [15, 15]
                m_ps = pp.tile([J, J], FP32, tag="pre_mm")
                nc.tensor.matmul(m_ps[:], mapfuc_T[:], acur[:])
                m_sb = cp.tile([J, J], FP32)
                nc.vector.tensor_copy(m_sb[:], m_ps[:])
                m_T = transpose15(m_sb, J, "m_T")

            # W = blockdiag_8(M^T)  [120, 120]; stationary operand so that
            # matmul out = W.T @ rhs applies M to each sample's 15 rows.
            w_sb = cp.tile([P, P], FP32)
            nc.gpsimd.memset(w_sb[:], 0.0)
            for s in range(S):
                nc.sync.dma_start(
                    w_sb[s * J:(s + 1) * J, s * J:(s + 1) * J], m_T[:]
                )

            # --- main streaming loop ---
            import os
            xin_bufs = int(os.environ.get("XIN_BUFS", "2"))
            yout_bufs = int(os.environ.get("YOUT_BUFS", "2"))
            dma_split = int(os.environ.get("DMA_SPLIT", "1"))
            alt_rings = int(os.environ.get("ALT_RINGS", "0"))
            with (
                tc.tile_pool(name="xin", bufs=xin_bufs) as xin_p,
                tc.tile_pool(name="yout", bufs=yout_bufs) as yout_p,
                tc.tile_pool(name="mm_psum", bufs=2,
                             space=bass.MemorySpace.PSUM) as mm_pp,
            ):
                def eng(i):
                    if not alt_rings:
                        return nc.sync, nc.scalar
                    return ((nc.sync, nc.scalar) if i % 2 == 0
                            else (nc.scalar, nc.sync))

                for i, g in enumerate(
                        [g for _ in range(repeats) for g in range(n_tiles)]):
                    if alt_rings == 2:
                        rings = (nc.sync, nc.scalar)
                        in_eng = out_eng = None
                    else:
                        in_eng, out_eng = eng(i)
                    xt = xin_p.tile([P, COLS], FP32)
                    step = COLS // dma_split
                    for d in range(dma_split):
                        e = rings[d % 2] if alt_rings == 2 else in_eng
                        e.dma_start(
                            xt[:, d * step:(d + 1) * step],
                            x[g * P:(g + 1) * P, d * step:(d + 1) * step])
                    if int(os.environ.get("IN_ONLY", "0")):
                        continue
                    if int(os.environ.get("COPY_ONLY", "0")):
                        for d in range(dma_split):
                            e = rings[(d + 1) % 2] if alt_rings == 2 else out_eng
                            e.dma_start(
                                y[g * P:(g + 1) * P, d * step:(d + 1) * step],
                                xt[:, d * step:(d + 1) * step])
                        continue
                    yt = yout_p.tile([P, COLS], FP32)
                    for h in range(NGROUP):
                        ps = mm_pp.tile([P, GROUP * CHUNK], FP32)
                        for c in range(GROUP):
                            lo = (h * GROUP + c) * CHUNK
                            nc.tensor.matmul(
                                ps[:, c * CHUNK:(c + 1) * CHUNK],
                                w_sb[:],
                                xt[:, lo:lo + CHUNK],
                            )
                        nc.scalar.activation(
                            yt[:, h * GROUP * CHUNK:(h + 1) * GROUP * CHUNK],
                            ps[:], AF.Sigmoid,
                        )
                    for d in range(dma_split):
                        e = rings[(d + 1) % 2] if alt_rings == 2 else out_eng
                        e.dma_start(
                            y[g * P:(g + 1) * P, d * step:(d + 1) * step],
                            yt[:, d * step:(d + 1) * step])

    nc.compile()
    return nc


def shard_inputs(inputs):
    """Host-side prep: per-core x re-layout + replicated tiny weights."""
    nf = np.ascontiguousarray(np.asarray(inputs["new_features"], dtype=np.float32))
    small = {
        "a_init": np.ascontiguousarray(np.asarray(inputs["A_init"], np.float32)),
        "a_change": np.ascontiguousarray(np.asarray(inputs["A_change"], np.float32)),
        "hidden": np.ascontiguousarray(np.asarray(inputs["Hidden"], np.float32)),
        "sigma": np.ascontiguousarray(np.asarray(inputs["sigma"], np.float32)),
        "kern": np.ascontiguousarray(np.asarray(inputs["kernel"], np.float32)),
        "bias_w": np.ascontiguousarray(np.asarray(inputs["bias"], np.float32)),
    }
    in_maps = []
    for c in range(N_CORES):
        shard = nf[c * SPC:(c + 1) * SPC]
        xc = np.ascontiguousarray(
            shard.reshape(G, T, S, J, F).transpose(0, 2, 3, 1, 4)
        ).reshape(G * P, COLS)
        in_maps.append({"x": xc, **small})
    return in_maps


def unshard_output(results):
    outs = []
    for c in range(N_CORES):
        yc = results[c]["y"]
        out = np.ascontiguousarray(
            yc.reshape(G, S, J, T, F).transpose(0, 3, 1, 2, 4)
        ).reshape(SPC, FEAT)
        outs.append(out)
    return np.concatenate(outs, axis=0)


def kernel(**inputs):
    if "nc" not in _CACHE:
        _CACHE["nc"] = build_nc()
    nc = _CACHE["nc"]
    in_maps = shard_inputs(inputs)
    res = run_bass_kernel_spmd(
        nc, in_maps, core_ids=list(range(N_CORES)),
        trace=TRACE, **TRACE_KWARGS,
    )
    _CACHE["last_result"] = res
    return unshard_output(res.results)
